# revision 1
# baseline (speedup 1.0000x reference)
"""Spatially-varying Gabor filter bank (31x31, per-pixel theta/freq) on 8 TRN2 cores.

Strategy
--------
Only 180*20 = 3600 distinct Gabor kernels exist (theta/freq are small ints), and
the whole kernel family is input-independent.  Host precomputes (in f64):
  * a 96-vector bf16 "cascade" basis Bm for the family (quantization-aware:
    top-80 SVD directions quantized to bf16, then 16 more bf16 directions
    fitted to the residual) and a rank-32 bf16 basis Bl for the low-order
    correction stream,
  * per-(theta,freq) combination coefficients, stored as a 3600x256 bf16
    table: row = [hi(96+32 coefs as bf16) | lo(residual bf16)].

On device, per core (band of 37 output rows):
  conv:    C[r, n] = sum_k basis[k, r] * patch[k, n] via PE matmuls.  The fp32
           image is split into hi/lo bf16 streams (phi + plo == fp32 exactly to
           ~2^-18); phi runs against the 96-col main basis (PE cols 0..95) and
           plo against the 32-col correction basis (PE cols 96..127,
           concurrently via tile_position col-tiling).  K dim is processed in 8
           chunks of 124 taps (4 kernel rows x 31 cols); the im2col "z-tiles"
           are shared by all 8 output rows that touch the same 4 image rows.
  combine: val[n] = sum_r coef[idx_n, r] * C[r, n]; coefficients are gathered
           (dma_gather transpose) from the bf16 hi/lo table, summed to fp32 on
           DVE, multiplied into C, and reduced over partitions with an exact
           fp32 matmul against a shifted-identity column (which also places
           row i of the result at psum partition i%32).
  minmax:  global min/max include the untouched border pixels; local
           reductions + one 8-core AllReduce(max) on [max, -min].
  binarize: out = 100 * (v > t), t = 0.55*max + 0.45*min.

Host only slices/pads/reshapes inputs and reassembles the 8 output bands.
"""

import os
import numpy as np

import concourse.bass as bass
import concourse.bacc as bacc
import concourse.tile as tile
from concourse import mybir
from concourse.ap import AP
from concourse import bass_isa
from concourse.bass_utils import run_bass_kernel_spmd
from contextlib import ExitStack

# ---------------------------------------------------------------- problem geometry
H = W = 320
KSIZE = 31
PAD = 15                       # KSIZE//2
HOUT = H - KSIZE + 1           # 289 interior rows (i = 15..303)
WOUT = W - KSIZE + 1           # 289 interior cols
NCORES = 8
ROWS_PER_CORE = 37             # 8*37 = 296 >= 289; last core has 30 real rows
BAND_ROWS = 68                 # 37 + 31 image rows needed per core
NZ = 65                        # z-tile count: z = i + 4q, i<37, q<8
NQ = 8                         # K chunks
KC = 124                       # taps per chunk (4 dy * 31 dx), last chunk zero-padded
R_MAIN = 96
R_LO = 32
NPIX = ROWS_PER_CORE * WOUT    # 10693 pixels per core
NIDX_G = 2432                  # pixels per gather: 8 rows (2312) padded to %128
NG = 5                         # gathers per core (rows 0..7, 8..15, ..., 32..36)
IDXC = NIDX_G // 16            # idx columns per gather (152)
SIGMA = 6.0
GAMMA_0 = 1.0
GAMMA_DELTA = 0.6
BIG = 1.0e30

_f32 = mybir.dt.float32
_bf16 = mybir.dt.bfloat16
_i32 = mybir.dt.int32
_i16 = mybir.dt.int16


def _to_bf16(x):
    """Round-to-nearest-even fp32 -> bf16, returned as fp32 values."""
    x32 = np.asarray(x, np.float32)
    u = x32.view(np.uint32)
    return (((u + 0x7FFF + ((u >> 16) & 1)) & 0xFFFF0000).astype(np.uint32)).view(np.float32)


def _build_lut_f64():
    """Exact kernel family K[theta, freq] -> [3600, 961] in f64."""
    half = KSIZE // 2
    r = np.arange(-half, half + 1, dtype=np.float64)
    yy, xx = np.meshgrid(r, r, indexing="ij")
    th = np.arange(180, dtype=np.float64) / 180.0 * np.pi
    fr = 0.025 + 0.0015 * np.arange(20, dtype=np.float64)
    ct, st = np.cos(th), np.sin(th)
    x_t = xx[None] * ct[:, None, None] + yy[None] * st[:, None, None]
    y_t = -xx[None] * st[:, None, None] + yy[None] * ct[:, None, None]
    gamma = GAMMA_0 + GAMMA_DELTA * np.abs(y_t) / half
    env = np.exp(-(x_t**2 + (gamma * y_t) ** 2) / (2.0 * SIGMA**2))
    w = 2.0 * np.pi * (1.0 + y_t / (3.0 * half)) * x_t
    K = env[:, None] * np.cos(fr[None, :, None, None] * w[:, None])
    return K.reshape(3600, KSIZE * KSIZE)


def _solve(B, M):
    """coef minimizing ||M - coef @ B||_F  (B [R,961], M [N,961]) -> [N, R]."""
    return np.linalg.lstsq(B.T, M.T, rcond=None)[0].T


def _cascade(widths, M):
    """Quantization-aware bf16 basis: blocks of SVD directions of the running
    residual, each quantized to bf16.  Returns (B [sum(widths), 961] bf16-exact
    f64, coef [N, sum(widths)] f64)."""
    blocks = []
    resid = M.copy()
    coef = None
    for wdt in widths:
        _, _, vt = np.linalg.svd(resid, full_matrices=False)
        blocks.append(_to_bf16(vt[:wdt].astype(np.float32)).astype(np.float64))
        Ball = np.vstack(blocks)
        coef = _solve(Ball, M)
        resid = M - coef @ Ball
    return np.vstack(blocks), coef


_CONSTS = None


def _build_constants():
    global _CONSTS
    if _CONSTS is not None:
        return _CONSTS
    K = _build_lut_f64()
    Bm, coef_m = _cascade((80, 16), K)       # [96, 961], [3600, 96]
    Bl, coef_l = _cascade((24, 8), K)        # [32, 961], [3600, 32]

    def chunked(B, rwidth):
        # [KC, NQ, rwidth] bf16; chunk q holds taps 124q .. 124q+123 (0 beyond 960)
        out = np.zeros((KC, NQ, rwidth), np.float32)
        for q in range(NQ):
            for p in range(KC):
                t = q * KC + p
                if t < KSIZE * KSIZE:
                    out[p, q, :] = B[:, t]
        import ml_dtypes
        return out.astype(ml_dtypes.bfloat16)

    bmain = chunked(Bm, R_MAIN)
    blo = chunked(Bl, R_LO)

    coef = np.concatenate([coef_m, coef_l], axis=1).astype(np.float32)  # [3600, 128]
    chi = _to_bf16(coef)
    clo = _to_bf16(coef - chi)
    import ml_dtypes
    table = np.concatenate([chi, clo], axis=1).astype(ml_dtypes.bfloat16)  # [3600, 256]
    _CONSTS = (bmain, blo, table)
    return _CONSTS


_STAGE = int(os.environ.get("GABOR_STAGE", "4"))  # dev bisect knob; 4 = full


def _build_program():
    """Build the SPMD Bass program (one NeuronCore's view)."""
    nc = bacc.Bacc("TRN2", target_bir_lowering=False, debug=False,
                   enable_asserts=True, num_devices=NCORES)

    # ---- DRAM parameters (per-core values supplied via in_maps)
    fband_d = nc.dram_tensor("fband", [BAND_ROWS, W], _f32, kind="ExternalInput").ap()
    extra_d = nc.dram_tensor("extra", [16, W], _f32, kind="ExternalInput").ap()
    thw_d = nc.dram_tensor("thw", [16, NG * IDXC], _i32, kind="ExternalInput").ap()
    fhw_d = nc.dram_tensor("fhw", [16, NG * IDXC], _i32, kind="ExternalInput").ap()
    rmask_d = nc.dram_tensor("rmask", [ROWS_PER_CORE, 1], _f32, kind="ExternalInput").ap()
    emask_d = nc.dram_tensor("emask", [16, 1], _f32, kind="ExternalInput").ap()
    bmain_d = nc.dram_tensor("bmain", [KC, NQ, R_MAIN], _bf16, kind="ExternalInput").ap()
    blo_d = nc.dram_tensor("blo", [KC, NQ, R_LO], _bf16, kind="ExternalInput").ap()
    table_d = nc.dram_tensor("table", [3600, 256], _bf16, kind="ExternalInput").ap()
    outb_d = nc.dram_tensor("out_band", [ROWS_PER_CORE, W], _f32, kind="ExternalOutput").ap()
    oute_d = nc.dram_tensor("out_extra", [16, W], _f32, kind="ExternalOutput").ap()

    with tile.TileContext(nc) as tc, ExitStack() as ctx:
        konst = ctx.enter_context(tc.tile_pool(name="konst", bufs=1))
        work = ctx.enter_context(tc.tile_pool(name="work", bufs=1))
        ptile = ctx.enter_context(tc.tile_pool(name="ptile", bufs=3))
        cpool = ctx.enter_context(tc.tile_pool(name="cpool", bufs=3, space="PSUM"))
        vpool = ctx.enter_context(tc.tile_pool(name="vpool", bufs=2, space="PSUM"))
        mpool = ctx.enter_context(tc.tile_pool(name="mpool", bufs=1, space="PSUM"))
        dpool = ctx.enter_context(tc.tile_pool(name="dram", bufs=1, space="DRAM"))

        # ---- load inputs / constants
        bandf = konst.tile([BAND_ROWS, W], _f32)
        nc.sync.dma_start(out=bandf, in_=fband_d)
        extra = konst.tile([16, W], _f32)
        nc.sync.dma_start(out=extra, in_=extra_d)
        bmain = konst.tile([KC, NQ, R_MAIN], _bf16)
        nc.sync.dma_start(out=bmain, in_=bmain_d)
        blo = konst.tile([KC, NQ, R_LO], _bf16)
        nc.sync.dma_start(out=blo, in_=blo_d)
        rmask = konst.tile([ROWS_PER_CORE, 1], _f32)
        nc.sync.dma_start(out=rmask, in_=rmask_d)
        emask = konst.tile([16, 1], _f32)
        nc.sync.dma_start(out=emask, in_=emask_d)

        _sub = float(os.environ.get("GABOR_SUBSTAGE", "9"))
        # ---- hi/lo split of the image band (exact: bhi + blo_ == bandf to ~2^-18)
        bhi = konst.tile([BAND_ROWS, W], _bf16)
        if _sub < 0.2:
            nc.vector.memset(bhi, 0.0)
        nc.vector.tensor_copy(bhi, bandf)
        bhi32 = work.tile([BAND_ROWS, W], _f32)
        nc.vector.tensor_copy(bhi32, bhi)
        bres = work.tile([BAND_ROWS, W], _f32)
        nc.vector.tensor_tensor(bres, bandf, bhi32, op=mybir.AluOpType.subtract)
        blo_ = konst.tile([BAND_ROWS, W], _bf16)
        nc.vector.tensor_copy(blo_, bres)

        # bounce hi/lo bands through DRAM so the im2col DMA can read overlapping
        # windows with an arbitrary-stride (DRAM-side) access pattern
        bhi_dr = dpool.tile([BAND_ROWS, W], _bf16)
        blo_dr = dpool.tile([BAND_ROWS, W], _bf16)
        nc.sync.dma_start(out=bhi_dr, in_=bhi)
        nc.sync.dma_start(out=blo_dr, in_=blo_)

        # ---- im2col z-tiles: t[dy*31+dx, z, j] = band[z+dy, j+dx]
        thi = konst.tile([KC, NZ, WOUT], _bf16)
        tlo = konst.tile([KC, NZ, WOUT], _bf16)
        nc.gpsimd.memset(thi, 0.0)
        nc.gpsimd.memset(tlo, 0.0)
        for t_sb, b_dr in (((thi, bhi_dr), (tlo, blo_dr)) if _sub >= 0.4 else ()):
            for dy in range(4):
                src = AP(b_dr[:].tensor, dy * W, [[1, KSIZE], [W, NZ], [1, WOUT]])
                nc.sync.dma_start(out=t_sb[dy * KSIZE:(dy + 1) * KSIZE, :, :], in_=src)

        # ---- per-pixel coefficient gathers (one per 8-row i-tile)
        idx32 = work.tile([16, NG * IDXC], _i32)
        thw = work.tile([16, NG * IDXC], _i32)
        fhw = work.tile([16, NG * IDXC], _i32)
        nc.sync.dma_start(out=thw, in_=thw_d)
        nc.sync.dma_start(out=fhw, in_=fhw_d)
        nc.vector.tensor_scalar_mul(idx32, thw, 20)
        nc.vector.tensor_tensor(idx32, idx32, fhw, op=mybir.AluOpType.add)
        idxs = work.tile([128, NG * IDXC], _i16)
        nc.gpsimd.memset(idxs, 0)
        nc.vector.tensor_copy(idxs[0:16, :], idx32.bitcast(_i16)[:, 0:2 * (NG * IDXC):2])
        # the gather's tx Q7 core reads its copy of the indices via partitions 16..31
        nc.sync.dma_start(out=idxs[16:32, :], in_=idxs[0:16, :])
        coefw = konst.tile([128, NG, 2, NIDX_G], _bf16)
        nc.gpsimd.memset(coefw, 0.0)
        for g in range(int(os.environ.get("GABOR_NGATHER", NG)) if _sub >= 0.6 else 0):
            nc.gpsimd.dma_gather(coefw[:, g, :, :], table_d, idxs[:, g * IDXC:(g + 1) * IDXC],
                                 num_idxs=NIDX_G, num_idxs_reg=NIDX_G,
                                 elem_size=256, transpose=True, single_packet=False)

        # ---- shifted-eye (for the fp32 partition-reduction matmul) and ones row
        eye = konst.tile([128, 63], _f32)
        nc.vector.memset(eye, 0.0)
        nc.vector.memset(eye[:, 31:32], 1.0)
        onesrow = konst.tile([1, 128], _f32)
        nc.vector.memset(onesrow, 1.0)

        # ---- main conv + combine loop
        vals = konst.tile([ROWS_PER_CORE, WOUT], _f32)
        nc.vector.memset(vals, 0.0)
        vps = {}
        for ri in range(ROWS_PER_CORE if _STAGE >= 2 else 0):
            g, m = divmod(ri, 32)
            Cfull = cpool.tile([128, 512], _f32, tag="Cps", name=f"C{ri}")
            C = Cfull[:, 0:WOUT]
            for q in range(NQ):
                z = ri + 4 * q
                nc.tensor.matmul(C[0:R_MAIN, :], lhsT=bmain[:, q, :], rhs=thi[:, z, :],
                                 start=(q == 0), stop=(q == NQ - 1))
                nc.tensor.matmul(C[96:96 + R_LO, :], lhsT=blo[:, q, :], rhs=tlo[:, z, :],
                                 start=(q == 0), stop=(q == NQ - 1), tile_position=(0, 96),
                                 skip_group_check=True)
            # s = coef_hi + coef_lo (fp32), P = C * s
            s = ptile.tile([128, WOUT], _f32, tag="s")
            gg, rloc = divmod(ri, 8)
            n0 = rloc * WOUT
            nc.vector.tensor_tensor(s, coefw[:, gg, 0, n0:n0 + WOUT],
                                    coefw[:, gg, 1, n0:n0 + WOUT],
                                    op=mybir.AluOpType.add)
            P = ptile.tile([128, WOUT], _f32, tag="P")
            nc.vector.tensor_tensor(P, C, s, op=mybir.AluOpType.mult)
            # val row -> psum partition m of group g (exact fp32 reduction)
            if g not in vps:
                vps[g] = vpool.tile([32, 512], _f32, tag="vps", name=f"vps{g}")[:, 0:WOUT]
            last_in_group = (ri == ROWS_PER_CORE - 1) or (m == 31)
            nc.tensor.matmul(vps[g], lhsT=eye[:, 31 - m:63 - m], rhs=P,
                             start=(m == 0), stop=last_in_group)
            if last_in_group:
                nrows = m + 1
                nc.vector.tensor_copy(vals[32 * g:32 * g + nrows, :], vps[g][0:nrows, :])
                del vps[g]

        if _STAGE < 3:
            out_band = work.tile([ROWS_PER_CORE, W], _f32)
            nc.vector.memset(out_band, 0.0)
            nc.vector.tensor_copy(out_band[:, PAD:PAD + WOUT], vals)
            out_extra = work.tile([16, W], _f32)
            nc.vector.memset(out_extra, 0.0)
            nc.sync.dma_start(out=outb_d, in_=out_band)
            nc.sync.dma_start(out=oute_d, in_=out_extra)
        _s3 = _STAGE >= 3
        if _s3:
            # ---- border strips (full-width rows are output rows; cols 0..14 & 304..319)
            bl = work.tile([ROWS_PER_CORE, PAD], _f32)
            br = work.tile([ROWS_PER_CORE, 16], _f32)
            nc.sync.dma_start(out=bl, in_=bandf[PAD:PAD + ROWS_PER_CORE, 0:PAD])
            nc.sync.dma_start(out=br, in_=bandf[PAD:PAD + ROWS_PER_CORE, W - 16:W])

            # ---- masked local min/max
            # offmax = (rmask-1)*BIG  (0 for valid rows, -BIG for pad rows)
            offmax = work.tile([ROWS_PER_CORE, 1], _f32)
            nc.vector.tensor_scalar(offmax, rmask, BIG, -BIG,
                                    op0=mybir.AluOpType.mult, op1=mybir.AluOpType.add)
            nrmask = work.tile([ROWS_PER_CORE, 1], _f32)
            nc.vector.tensor_scalar_mul(nrmask, rmask, -1.0)
            eoffmax = work.tile([16, 1], _f32)
            nc.vector.tensor_scalar(eoffmax, emask, BIG, -BIG,
                                    op0=mybir.AluOpType.mult, op1=mybir.AluOpType.add)
            nemask = work.tile([16, 1], _f32)
            nc.vector.tensor_scalar_mul(nemask, emask, -1.0)

            cand_max = work.tile([ROWS_PER_CORE, 4], _f32)
            cand_min = work.tile([ROWS_PER_CORE, 4], _f32)  # holds NEGATED minima
            nc.vector.memset(cand_max, -BIG)
            nc.vector.memset(cand_min, -BIG)

            tmp = work.tile([ROWS_PER_CORE, 1], _f32)
            for col, (tens, msk, nmsk, off) in enumerate((
                    (vals, rmask, nrmask, offmax),
                    (bl, rmask, nrmask, offmax),
                    (br, rmask, nrmask, offmax),
                    (extra, emask, nemask, eoffmax))):
                nr = tens.shape[0]
                nc.vector.tensor_reduce(tmp[0:nr, :], tens[:, :], axis=mybir.AxisListType.X,
                                        op=mybir.AluOpType.max)
                nc.vector.tensor_scalar(cand_max[0:nr, col:col + 1], tmp[0:nr, :], msk[0:nr, :],
                                        off[0:nr, :], op0=mybir.AluOpType.mult,
                                        op1=mybir.AluOpType.add)
                nc.vector.tensor_reduce(tmp[0:nr, :], tens[:, :], axis=mybir.AxisListType.X,
                                        op=mybir.AluOpType.min)
                nc.vector.tensor_scalar(cand_min[0:nr, col:col + 1], tmp[0:nr, :], nmsk[0:nr, :],
                                        off[0:nr, :], op0=mybir.AluOpType.mult,
                                        op1=mybir.AluOpType.add)

            comb = work.tile([ROWS_PER_CORE, 2], _f32)
            nc.vector.tensor_reduce(comb[:, 0:1], cand_max[:, :], axis=mybir.AxisListType.X,
                                    op=mybir.AluOpType.max)
            nc.vector.tensor_reduce(comb[:, 1:2], cand_min[:, :], axis=mybir.AxisListType.X,
                                    op=mybir.AluOpType.max)
            comb2 = work.tile([ROWS_PER_CORE, 2], _f32)
            nc.gpsimd.partition_all_reduce(comb2, comb, channels=ROWS_PER_CORE,
                                           reduce_op=bass_isa.ReduceOp.max)

            if _STAGE == 3:
                out_band = work.tile([ROWS_PER_CORE, W], _f32)
                nc.vector.memset(out_band, 0.0)
                nc.vector.tensor_copy(out_band[:, PAD:PAD + WOUT], vals)
                nc.vector.tensor_copy(out_band[:, 0:2], comb2[:, 0:2])
                out_extra = work.tile([16, W], _f32)
                nc.vector.memset(out_extra, 0.0)
                nc.sync.dma_start(out=outb_d, in_=out_band)
                nc.sync.dma_start(out=oute_d, in_=out_extra)
        _s4 = _STAGE >= 4
        if _s4:
            # ---- 8-core AllReduce(max) on [local_max, -local_min]
            cc_in = dpool.tile([1, 2], _f32)
            cc_out = nc.dram_tensor("cc_out", [1, 2], _f32, addr_space="Shared").ap()
            nc.sync.dma_start(out=cc_in, in_=comb2[0:1, :])
            nc.gpsimd.collective_compute("AllReduce", mybir.AluOpType.max,
                                         replica_groups=[list(range(NCORES))],
                                         ins=[cc_in[:]], outs=[cc_out])
            gmm = work.tile([1, 2], _f32)
            nc.sync.dma_start(out=gmm, in_=cc_out)

            # ---- threshold t = 0.55*max + 0.45*min = 0.55*gmm[0] - 0.45*gmm[1]
            t_a = work.tile([1, 1], _f32)
            t_b = work.tile([1, 1], _f32)
            nc.vector.tensor_scalar_mul(t_a, gmm[0:1, 0:1], 0.55)
            nc.vector.tensor_scalar_mul(t_b, gmm[0:1, 1:2], 0.45)
            t00 = work.tile([1, 1], _f32)
            nc.vector.tensor_tensor(t00, t_a, t_b, op=mybir.AluOpType.subtract)
            tb_ps = mpool.tile([128, 1], _f32)
            nc.tensor.matmul(tb_ps, lhsT=onesrow, rhs=t00, start=True, stop=True)
            tb = work.tile([128, 1], _f32)
            nc.vector.tensor_copy(tb, tb_ps)

            # ---- binarize: 100 * (v > t)
            out_band = work.tile([ROWS_PER_CORE, W], _f32)
            nc.vector.tensor_scalar(out_band[:, PAD:PAD + WOUT], vals, tb[0:ROWS_PER_CORE, :],
                                    100.0, op0=mybir.AluOpType.is_gt, op1=mybir.AluOpType.mult)
            nc.vector.tensor_scalar(out_band[:, 0:PAD], bl, tb[0:ROWS_PER_CORE, :], 100.0,
                                    op0=mybir.AluOpType.is_gt, op1=mybir.AluOpType.mult)
            nc.vector.tensor_scalar(out_band[:, W - 16:W], br, tb[0:ROWS_PER_CORE, :], 100.0,
                                    op0=mybir.AluOpType.is_gt, op1=mybir.AluOpType.mult)
            out_extra = work.tile([16, W], _f32)
            nc.vector.tensor_scalar(out_extra, extra, tb[0:16, :], 100.0,
                                    op0=mybir.AluOpType.is_gt, op1=mybir.AluOpType.mult)
            nc.sync.dma_start(out=outb_d, in_=out_band)
            nc.sync.dma_start(out=oute_d, in_=out_extra)

    nc.compile()
    return nc


_PROGRAM = None


def _get_program():
    global _PROGRAM
    if _PROGRAM is None:
        _PROGRAM = _build_program()
    return _PROGRAM


def _make_in_maps(fprint, freq_map, theta_map):
    bmain, blo, table = _build_constants()
    fprint = np.asarray(fprint, np.float32)
    freq_map = np.asarray(freq_map, np.int32)
    theta_map = np.asarray(theta_map, np.int32)

    in_maps = []
    for c in range(NCORES):
        r0 = ROWS_PER_CORE * c          # first output row (interior index)
        fband = np.zeros((BAND_ROWS, W), np.float32)
        lo = r0
        hi = min(r0 + BAND_ROWS, H)
        fband[0:hi - lo] = fprint[lo:hi]

        extra = np.zeros((16, W), np.float32)
        if c == 0:
            extra[:] = fprint[0:16]
        elif c == NCORES - 1:
            extra[:] = fprint[H - 16:H]

        # wrapped theta/freq for the gathers' index layout (per 8-row i-tile)
        nreal = min(ROWS_PER_CORE, HOUT - r0)
        thw = np.zeros((16, NG * IDXC), np.int32)
        fhw = np.zeros((16, NG * IDXC), np.int32)
        for g in range(NG):
            th = np.zeros(NIDX_G, np.int32)
            fq = np.zeros(NIDX_G, np.int32)
            lo_r = 8 * g
            hi_r = min(lo_r + 8, nreal)
            if hi_r > lo_r:
                nrw = (hi_r - lo_r) * WOUT
                th[0:nrw] = theta_map[PAD + r0 + lo_r:PAD + r0 + hi_r,
                                      PAD:PAD + WOUT].reshape(-1)
                fq[0:nrw] = freq_map[PAD + r0 + lo_r:PAD + r0 + hi_r,
                                     PAD:PAD + WOUT].reshape(-1)
            thw[:, g * IDXC:(g + 1) * IDXC] = th.reshape(IDXC, 16).T
            fhw[:, g * IDXC:(g + 1) * IDXC] = fq.reshape(IDXC, 16).T

        rmask = np.zeros((ROWS_PER_CORE, 1), np.float32)
        rmask[0:nreal] = 1.0
        emask = np.zeros((16, 1), np.float32)
        if c == 0:
            emask[0:15] = 1.0   # row 15 of extra is an interior row; exclude
        elif c == NCORES - 1:
            emask[:] = 1.0

        in_maps.append({
            "fband": fband, "extra": extra, "thw": thw, "fhw": fhw,
            "rmask": rmask, "emask": emask,
            "bmain": bmain, "blo": blo, "table": table,
        })
    return in_maps


def _assemble(results, fprint_dtype=np.float32):
    out = np.zeros((H, W), np.float32)
    for c in range(NCORES):
        r0 = ROWS_PER_CORE * c
        nreal = min(ROWS_PER_CORE, HOUT - r0)
        band = np.asarray(results[c]["out_band"])
        out[PAD + r0:PAD + r0 + nreal, :] = band[0:nreal, :]
    out[0:PAD, :] = np.asarray(results[0]["out_extra"])[0:PAD, :]
    out[H - 16:H, :] = np.asarray(results[NCORES - 1]["out_extra"])
    return out.astype(fprint_dtype)


def kernel(fprint, freq_map, theta_map, _trace=False):
    nc = _get_program()
    in_maps = _make_in_maps(fprint, freq_map, theta_map)
    res = run_bass_kernel_spmd(nc, in_maps, list(range(NCORES)), trace=_trace)
    out = _assemble(res.results)
    if _trace:
        kernel.last_exec_time_ns = res.exec_time_ns
        kernel.last_results = res
    return out



# revision 6
# speedup vs baseline: 2.6707x; 2.6707x over previous
"""Spatially-varying Gabor filter bank (31x31, per-pixel theta/freq) on 8 TRN2 cores.

v2 design
---------
Only 180*20 = 3600 distinct Gabor kernels exist; host precomputes a rank-64
fp16 basis B (quantization-aware 56+8 SVD cascade) for the family and a
per-(theta,freq) coefficient table, stored bf16 as [hi64 | lo64] per row
(hi+lo sums to ~fp24 coefficient precision).

Device, per core (band of 37 output rows x 289 interior cols):
  im2col: z-tiles t[dy*31+dx, z, j] = band16[z+dy, j+dx] (fp16), built by
          SWDGE (gpsimd) DMA so descriptors spread over all 16 SDMA engines
          (the HWDGE sync ring serializes small descriptors onto one engine).
  conv:   C[r, n] = sum_k basis[k, r] * patch[k, n] on the PE; the fp16 image
          stream needs no hi/lo split (quantization error ~0.03 abs vs the
          ~0.4 margin to the binarization threshold).  The basis is stored
          with duplicated columns [B | B] -> psum C[64+r] == C[r], so the
          bf16 hi and lo coefficient halves multiply separate psum copies and
          the partition-reduce applies coef_hi + coef_lo exactly - no DVE add.
  combine: per-pixel coefs arrive via gpsimd dma_gather (transpose) keyed by
          idx = 20*theta + freq computed on-device; P = C * coef on DVE; the
          128-partition reduction is an fp32 matmul against a shifted-identity
          column placing row i at psum partition i%32.
  minmax/binarize: masked local min/max incl. borders, one 8-core
          AllReduce(max) on [max, -min], threshold 0.55max+0.45min, 100*(v>t).
"""

import os
import numpy as np

import concourse.bass as bass
import concourse.bacc as bacc
import concourse.tile as tile
from concourse import mybir
from concourse.ap import AP
from concourse import bass_isa
from concourse.bass_utils import run_bass_kernel_spmd
from contextlib import ExitStack

# ---------------------------------------------------------------- geometry
H = W = 320
KSIZE = 31
PAD = 15
HOUT = H - KSIZE + 1           # 289
WOUT = W - KSIZE + 1
NCORES = 8
ROWS_PER_CORE = 37
BAND_ROWS = 68
NZ = 65
NQ = 8
KC = 124
RANK = 64
NIDX_G = 2432
NG = 5
IDXC = NIDX_G // 16            # 152
SIGMA = 6.0
GAMMA_0 = 1.0
GAMMA_DELTA = 0.6
BIG = 1.0e30

_f32 = mybir.dt.float32
_bf16 = mybir.dt.bfloat16
_fp16 = mybir.dt.float16
_i32 = mybir.dt.int32
_i16 = mybir.dt.int16

_HOSTGATHER = os.environ.get("GV2_HOSTGATHER", "0") == "1"
_SINGLEPACKET = os.environ.get("GV2_SINGLEPACKET", "0") == "1"


def _to_bf16(x):
    x32 = np.asarray(x, np.float32)
    u = x32.view(np.uint32)
    return (((u + 0x7FFF + ((u >> 16) & 1)) & 0xFFFF0000).astype(np.uint32)).view(np.float32)


def _build_lut_f64():
    half = KSIZE // 2
    r = np.arange(-half, half + 1, dtype=np.float64)
    yy, xx = np.meshgrid(r, r, indexing="ij")
    th = np.arange(180, dtype=np.float64) / 180.0 * np.pi
    fr = 0.025 + 0.0015 * np.arange(20, dtype=np.float64)
    ct, st = np.cos(th), np.sin(th)
    x_t = xx[None] * ct[:, None, None] + yy[None] * st[:, None, None]
    y_t = -xx[None] * st[:, None, None] + yy[None] * ct[:, None, None]
    gamma = GAMMA_0 + GAMMA_DELTA * np.abs(y_t) / half
    env = np.exp(-(x_t**2 + (gamma * y_t) ** 2) / (2.0 * SIGMA**2))
    w = 2.0 * np.pi * (1.0 + y_t / (3.0 * half)) * x_t
    K = env[:, None] * np.cos(fr[None, :, None, None] * w[:, None])
    return K.reshape(3600, KSIZE * KSIZE)


def _cascade_fp16(widths, M):
    blocks = []
    resid = M.copy()
    coef = None
    for wdt in widths:
        _, _, vt = np.linalg.svd(resid, full_matrices=False)
        blocks.append(np.asarray(vt[:wdt], np.float32).astype(np.float16).astype(np.float64))
        Ball = np.vstack(blocks)
        coef = np.linalg.lstsq(Ball.T, M.T, rcond=None)[0].T
        resid = M - coef @ Ball
    return np.vstack(blocks), coef


_CONSTS = None


def _build_constants():
    global _CONSTS
    if _CONSTS is not None:
        return _CONSTS
    import ml_dtypes
    K = _build_lut_f64()
    B, coef = _cascade_fp16((56, 8), K)      # [64, 961], [3600, 64]

    # chunked, column-duplicated basis: [KC, NQ, 128] fp16
    bchunk = np.zeros((KC, NQ, 2 * RANK), np.float32)
    for q in range(NQ):
        for p in range(KC):
            t = q * KC + p
            if t < KSIZE * KSIZE:
                bchunk[p, q, 0:RANK] = B[:, t]
                bchunk[p, q, RANK:2 * RANK] = B[:, t]
    bchunk = bchunk.astype(np.float16)

    chi = _to_bf16(coef.astype(np.float32))
    clo = _to_bf16(coef.astype(np.float32) - chi)
    table = np.concatenate([chi, clo], axis=1).astype(ml_dtypes.bfloat16)  # [3600, 128]
    _CONSTS = (bchunk, table)
    return _CONSTS


def _build_program():
    nc = bacc.Bacc("TRN2", target_bir_lowering=False, debug=False,
                   enable_asserts=True, num_devices=NCORES)

    fb16_d = nc.dram_tensor("fb16", [BAND_ROWS + 1, W], _fp16, kind="ExternalInput").ap()
    fband_d = nc.dram_tensor("fband", [BAND_ROWS, W], _f32, kind="ExternalInput").ap()
    extra_d = nc.dram_tensor("extra", [16, W], _f32, kind="ExternalInput").ap()
    rmask_d = nc.dram_tensor("rmask", [ROWS_PER_CORE, 1], _f32, kind="ExternalInput").ap()
    emask_d = nc.dram_tensor("emask", [16, 1], _f32, kind="ExternalInput").ap()
    bchunk_d = nc.dram_tensor("bchunk", [KC, NQ, 2 * RANK], _fp16, kind="ExternalInput").ap()
    if _HOSTGATHER:
        scoef_d = nc.dram_tensor("scoef", [128, NG * NIDX_G], _bf16, kind="ExternalInput").ap()
    else:
        thw_d = nc.dram_tensor("thw", [16, NG * IDXC], _i32, kind="ExternalInput").ap()
        fhw_d = nc.dram_tensor("fhw", [16, NG * IDXC], _i32, kind="ExternalInput").ap()
        table_d = nc.dram_tensor("table", [3600, 2 * RANK], _bf16, kind="ExternalInput").ap()
    outb_d = nc.dram_tensor("out_band", [ROWS_PER_CORE, W], _f32, kind="ExternalOutput").ap()
    oute_d = nc.dram_tensor("out_extra", [16, W], _f32, kind="ExternalOutput").ap()

    with tile.TileContext(nc) as tc, ExitStack() as ctx:
        konst = ctx.enter_context(tc.tile_pool(name="konst", bufs=1))
        work = ctx.enter_context(tc.tile_pool(name="work", bufs=1))
        ptile = ctx.enter_context(tc.tile_pool(name="ptile", bufs=3))
        cpool = ctx.enter_context(tc.tile_pool(name="cpool", bufs=3, space="PSUM"))
        vpool = ctx.enter_context(tc.tile_pool(name="vpool", bufs=2, space="PSUM"))
        mpool = ctx.enter_context(tc.tile_pool(name="mpool", bufs=1, space="PSUM"))
        dpool = ctx.enter_context(tc.tile_pool(name="dram", bufs=1, space="DRAM"))

        # ---- small inputs on the sync ring
        bandf = konst.tile([BAND_ROWS, W], _f32)
        nc.sync.dma_start(out=bandf, in_=fband_d)
        extra = konst.tile([16, W], _f32)
        nc.sync.dma_start(out=extra, in_=extra_d)
        rmask = konst.tile([ROWS_PER_CORE, 1], _f32)
        nc.sync.dma_start(out=rmask, in_=rmask_d)
        emask = konst.tile([16, 1], _f32)
        nc.sync.dma_start(out=emask, in_=emask_d)

        # ---- per-pixel coefficient indices (device path)
        coefw = konst.tile([128, NG, 1, NIDX_G], _bf16)
        if not _HOSTGATHER:
            thw = work.tile([16, NG * IDXC], _i32)
            fhw = work.tile([16, NG * IDXC], _i32)
            nc.sync.dma_start(out=thw, in_=thw_d)
            nc.sync.dma_start(out=fhw, in_=fhw_d)
            idx32 = work.tile([16, NG * IDXC], _i32)
            nc.vector.tensor_scalar_mul(idx32, thw, 20)
            nc.vector.tensor_tensor(idx32, idx32, fhw, op=mybir.AluOpType.add)
            idxs = work.tile([128, NG * IDXC], _i16)
            nc.gpsimd.memset(idxs, 0)
            nc.vector.tensor_copy(idxs[0:16, :], idx32.bitcast(_i16)[:, 0:2 * (NG * IDXC):2])
            nc.sync.dma_start(out=idxs[16:32, :], in_=idxs[0:16, :])

        # ---- gpsimd (SWDGE) queue: basis, im2col chunks, then gathers
        bchunk = konst.tile([KC, NQ, 2 * RANK], _fp16)
        nc.gpsimd.dma_start(out=bchunk, in_=bchunk_d)

        # im2col as flat contiguous runs: t[31*dy+dx, z*W + j] = band[z+dy, j+dx].
        # Per (dy, dx) the 65 z-rows are one contiguous DRAM run starting at
        # dy*W + dx (the dx shift folds into the start address; the rhs later
        # reads [:, z*W : z*W+289] so the overhang never enters a matmul).
        # 31 fat descriptors per call -> line-rate on the sync HWDGE ring.
        thi = konst.tile([KC, NZ, W], _fp16)
        ZSPLITS = ((0, 33), (33, 65))
        for z0, z1 in ZSPLITS:
            with nc.named_scope(f"imcol_z{z0}"):
                for dy in range(4):
                    src = AP(fb16_d.tensor, (z0 + dy) * W, [[1, KSIZE], [1, (z1 - z0) * W]])
                    nc.sync.dma_start(out=thi[dy * KSIZE:(dy + 1) * KSIZE, z0:z1, :], in_=src)

        if _HOSTGATHER:
            nc.gpsimd.dma_start(out=coefw.reshape([128, NG * NIDX_G]), in_=scoef_d)
        else:
            for g in range(NG):
                with nc.named_scope(f"gather{g}"):
                    nc.gpsimd.dma_gather(coefw[:, g, :, :], table_d,
                                         idxs[:, g * IDXC:(g + 1) * IDXC],
                                         num_idxs=NIDX_G, num_idxs_reg=NIDX_G,
                                         elem_size=2 * RANK, transpose=True,
                                         single_packet=_SINGLEPACKET)

        # ---- shifted-eye for the fp32 partition-reduction matmul; ones row
        eye = konst.tile([128, 63], _f32)
        nc.vector.memset(eye, 0.0)
        nc.vector.memset(eye[:, 31:32], 1.0)
        onesrow = konst.tile([1, 128], _f32)
        nc.vector.memset(onesrow, 1.0)

        # ---- conv + combine
        vals = konst.tile([ROWS_PER_CORE, WOUT], _f32)
        vps = {}
        with nc.named_scope("conv"):
            for ri in range(ROWS_PER_CORE):
                Cfull = cpool.tile([128, 512], _f32, tag="Cps", name=f"C{ri}")
                C = Cfull[:, 0:WOUT]
                for q in range(NQ):
                    z = ri + 4 * q
                    nc.tensor.matmul(C, lhsT=bchunk[:, q, :], rhs=thi[:, z, 0:WOUT],
                                     start=(q == 0), stop=(q == NQ - 1))
                gg, rloc = divmod(ri, 8)
                n0 = rloc * WOUT
                P = ptile.tile([128, WOUT], _f32, tag="P")
                nc.vector.tensor_tensor(P, C, coefw[:, gg, 0, n0:n0 + WOUT],
                                        op=mybir.AluOpType.mult)
                g32, m = divmod(ri, 32)
                if g32 not in vps:
                    vps[g32] = vpool.tile([32, 512], _f32, tag="vps", name=f"vps{g32}")[:, 0:WOUT]
                last_in_group = (ri == ROWS_PER_CORE - 1) or (m == 31)
                nc.tensor.matmul(vps[g32], lhsT=eye[:, 31 - m:63 - m], rhs=P,
                                 start=(m == 0), stop=last_in_group)
                if last_in_group:
                    nrows = m + 1
                    nc.vector.tensor_copy(vals[32 * g32:32 * g32 + nrows, :],
                                          vps[g32][0:nrows, :])
                    del vps[g32]

        # ---- border strips
        bl = work.tile([ROWS_PER_CORE, PAD], _f32)
        br = work.tile([ROWS_PER_CORE, 16], _f32)
        nc.sync.dma_start(out=bl, in_=bandf[PAD:PAD + ROWS_PER_CORE, 0:PAD])
        nc.sync.dma_start(out=br, in_=bandf[PAD:PAD + ROWS_PER_CORE, W - 16:W])

        # ---- masked local min/max
        with nc.named_scope("minmax"):
            offmax = work.tile([ROWS_PER_CORE, 1], _f32)
            nc.vector.tensor_scalar(offmax, rmask, BIG, -BIG,
                                    op0=mybir.AluOpType.mult, op1=mybir.AluOpType.add)
            nrmask = work.tile([ROWS_PER_CORE, 1], _f32)
            nc.vector.tensor_scalar_mul(nrmask, rmask, -1.0)
            eoffmax = work.tile([16, 1], _f32)
            nc.vector.tensor_scalar(eoffmax, emask, BIG, -BIG,
                                    op0=mybir.AluOpType.mult, op1=mybir.AluOpType.add)
            nemask = work.tile([16, 1], _f32)
            nc.vector.tensor_scalar_mul(nemask, emask, -1.0)

            cand_max = work.tile([ROWS_PER_CORE, 4], _f32)
            cand_min = work.tile([ROWS_PER_CORE, 4], _f32)  # NEGATED minima
            nc.vector.memset(cand_max, -BIG)
            nc.vector.memset(cand_min, -BIG)

            tmp = work.tile([ROWS_PER_CORE, 1], _f32)
            for col, (tens, msk, nmsk, off) in enumerate((
                    (vals, rmask, nrmask, offmax),
                    (bl, rmask, nrmask, offmax),
                    (br, rmask, nrmask, offmax),
                    (extra, emask, nemask, eoffmax))):
                nr = tens.shape[0]
                nc.vector.tensor_reduce(tmp[0:nr, :], tens[:, :], axis=mybir.AxisListType.X,
                                        op=mybir.AluOpType.max)
                nc.vector.tensor_scalar(cand_max[0:nr, col:col + 1], tmp[0:nr, :], msk[0:nr, :],
                                        off[0:nr, :], op0=mybir.AluOpType.mult,
                                        op1=mybir.AluOpType.add)
                nc.vector.tensor_reduce(tmp[0:nr, :], tens[:, :], axis=mybir.AxisListType.X,
                                        op=mybir.AluOpType.min)
                nc.vector.tensor_scalar(cand_min[0:nr, col:col + 1], tmp[0:nr, :], nmsk[0:nr, :],
                                        off[0:nr, :], op0=mybir.AluOpType.mult,
                                        op1=mybir.AluOpType.add)

            comb = work.tile([ROWS_PER_CORE, 2], _f32)
            nc.vector.tensor_reduce(comb[:, 0:1], cand_max[:, :], axis=mybir.AxisListType.X,
                                    op=mybir.AluOpType.max)
            nc.vector.tensor_reduce(comb[:, 1:2], cand_min[:, :], axis=mybir.AxisListType.X,
                                    op=mybir.AluOpType.max)
            comb2 = work.tile([ROWS_PER_CORE, 2], _f32)
            nc.gpsimd.partition_all_reduce(comb2, comb, channels=ROWS_PER_CORE,
                                           reduce_op=bass_isa.ReduceOp.max)

        # ---- 8-core AllReduce(max) on [local_max, -local_min]
        with nc.named_scope("allreduce"):
            cc_in = dpool.tile([1, 2], _f32)
            cc_out = nc.dram_tensor("cc_out", [1, 2], _f32, addr_space="Shared").ap()
            nc.sync.dma_start(out=cc_in, in_=comb2[0:1, :])
            nc.gpsimd.collective_compute("AllReduce", mybir.AluOpType.max,
                                         replica_groups=[list(range(NCORES))],
                                         ins=[cc_in[:]], outs=[cc_out])
            gmm = work.tile([1, 2], _f32)
            nc.sync.dma_start(out=gmm, in_=cc_out)

        # ---- threshold t = 0.55*max - 0.45*(-min); broadcast to 128 partitions
        with nc.named_scope("tail"):
            t_a = work.tile([1, 1], _f32)
            t_b = work.tile([1, 1], _f32)
            nc.vector.tensor_scalar_mul(t_a, gmm[0:1, 0:1], 0.55)
            nc.vector.tensor_scalar_mul(t_b, gmm[0:1, 1:2], 0.45)
            t00 = work.tile([1, 1], _f32)
            nc.vector.tensor_tensor(t00, t_a, t_b, op=mybir.AluOpType.subtract)
            tb_ps = mpool.tile([128, 1], _f32)
            nc.tensor.matmul(tb_ps, lhsT=onesrow, rhs=t00, start=True, stop=True)
            tb = work.tile([128, 1], _f32)
            nc.vector.tensor_copy(tb, tb_ps)

            out_band = work.tile([ROWS_PER_CORE, W], _f32)
            nc.vector.tensor_scalar(out_band[:, PAD:PAD + WOUT], vals, tb[0:ROWS_PER_CORE, :],
                                    100.0, op0=mybir.AluOpType.is_gt, op1=mybir.AluOpType.mult)
            nc.vector.tensor_scalar(out_band[:, 0:PAD], bl, tb[0:ROWS_PER_CORE, :], 100.0,
                                    op0=mybir.AluOpType.is_gt, op1=mybir.AluOpType.mult)
            nc.vector.tensor_scalar(out_band[:, W - 16:W], br, tb[0:ROWS_PER_CORE, :], 100.0,
                                    op0=mybir.AluOpType.is_gt, op1=mybir.AluOpType.mult)
            out_extra = work.tile([16, W], _f32)
            nc.vector.tensor_scalar(out_extra, extra, tb[0:16, :], 100.0,
                                    op0=mybir.AluOpType.is_gt, op1=mybir.AluOpType.mult)
            nc.sync.dma_start(out=outb_d, in_=out_band)
            nc.sync.dma_start(out=oute_d, in_=out_extra)

    nc.compile()
    return nc


_PROGRAM = None


def _get_program():
    global _PROGRAM
    if _PROGRAM is None:
        _PROGRAM = _build_program()
    return _PROGRAM


def _make_in_maps(fprint, freq_map, theta_map):
    bchunk, table = _build_constants()
    fprint = np.asarray(fprint, np.float32)
    freq_map = np.asarray(freq_map, np.int32)
    theta_map = np.asarray(theta_map, np.int32)

    in_maps = []
    for c in range(NCORES):
        r0 = ROWS_PER_CORE * c
        fband = np.zeros((BAND_ROWS, W), np.float32)
        lo = r0
        hi = min(r0 + BAND_ROWS, H)
        fband[0:hi - lo] = fprint[lo:hi]

        extra = np.zeros((16, W), np.float32)
        if c == 0:
            extra[:] = fprint[0:16]
        elif c == NCORES - 1:
            extra[:] = fprint[H - 16:H]

        nreal = min(ROWS_PER_CORE, HOUT - r0)
        idxmat = np.zeros((NG, NIDX_G), np.int64)
        thw = np.zeros((16, NG * IDXC), np.int32)
        fhw = np.zeros((16, NG * IDXC), np.int32)
        for g in range(NG):
            th = np.zeros(NIDX_G, np.int32)
            fq = np.zeros(NIDX_G, np.int32)
            lo_r = 8 * g
            hi_r = min(lo_r + 8, nreal)
            if hi_r > lo_r:
                nrw = (hi_r - lo_r) * WOUT
                th[0:nrw] = theta_map[PAD + r0 + lo_r:PAD + r0 + hi_r,
                                      PAD:PAD + WOUT].reshape(-1)
                fq[0:nrw] = freq_map[PAD + r0 + lo_r:PAD + r0 + hi_r,
                                     PAD:PAD + WOUT].reshape(-1)
            idxmat[g] = th.astype(np.int64) * 20 + fq
            thw[:, g * IDXC:(g + 1) * IDXC] = th.reshape(IDXC, 16).T
            fhw[:, g * IDXC:(g + 1) * IDXC] = fq.reshape(IDXC, 16).T

        rmask = np.zeros((ROWS_PER_CORE, 1), np.float32)
        rmask[0:nreal] = 1.0
        emask = np.zeros((16, 1), np.float32)
        if c == 0:
            emask[0:15] = 1.0
        elif c == NCORES - 1:
            emask[:] = 1.0

        fb16 = np.zeros((BAND_ROWS + 1, W), np.float16)
        fb16[0:hi - lo] = fprint[lo:hi].astype(np.float16)
        m = {
            "fb16": fb16,
            "fband": fband, "extra": extra,
            "rmask": rmask, "emask": emask, "bchunk": bchunk,
        }
        if _HOSTGATHER:
            # scoef[p, g*NIDX+s] = table[idx(g,s), p]
            gath = table[idxmat.reshape(-1)]              # [NG*NIDX, 128] bf16
            m["scoef"] = np.ascontiguousarray(gath.T)     # [128, NG*NIDX]
        else:
            m["thw"] = thw
            m["fhw"] = fhw
            m["table"] = table
        in_maps.append(m)
    return in_maps


def _assemble(results):
    out = np.zeros((H, W), np.float32)
    for c in range(NCORES):
        r0 = ROWS_PER_CORE * c
        nreal = min(ROWS_PER_CORE, HOUT - r0)
        band = np.asarray(results[c]["out_band"])
        out[PAD + r0:PAD + r0 + nreal, :] = band[0:nreal, :]
    out[0:PAD, :] = np.asarray(results[0]["out_extra"])[0:PAD, :]
    out[H - 16:H, :] = np.asarray(results[NCORES - 1]["out_extra"])
    return out


def kernel(fprint, freq_map, theta_map, _trace=False):
    nc = _get_program()
    in_maps = _make_in_maps(fprint, freq_map, theta_map)
    res = run_bass_kernel_spmd(nc, in_maps, list(range(NCORES)), trace=_trace)
    out = _assemble(res.results)
    if _trace:
        kernel.last_exec_time_ns = res.exec_time_ns
        kernel.last_results = res
    return out
